# revision 14
# baseline (speedup 1.0000x reference)
"""Trainium2 Bass kernel: causal self-attention with RoPE (B=4, T=2048, D=1024, H=16, Dh=64).

Sharding: 8 cores = 4 batches x 2 head-halves. Core c handles batch c//2 and
heads (c%2)*8 .. (c%2)*8+7 (feature columns (c%2)*512 .. +512 of Wq/Wk/Wv, and
the matching rows of Wo). Each core computes a partial output [T, D]; the host
sums the two partials per batch (row-sharded Wo reduction) and stacks batches.

On-chip layout: activations are kept transposed (features on partitions):
  xT [D, T] (spilled to DRAM), qT/kT [512, T], scoresT [s, t], attn_outT [512, T].
This makes every matmul contraction land on the partition dim with zero
transposes except one PE-transpose pass over x. The softmax denominator is
fused into the AV matmul via a ones-column appended to V (M=65), and the
causal mask is applied post-exp with a single tensor_mask per diagonal group.
"""

import os
import sys

for _p in ("/opt/trn_rl_repo", "/root/.axon_site/_ro/trn_rl_repo"):
    if os.path.isdir(_p) and _p not in sys.path:
        sys.path.append(_p)

import numpy as np

import bass_rust
import concourse.bass as bass
import concourse.mybir as mybir
import concourse.tile as tile
from concourse.vector_clock import ScopedClock

F32 = mybir.dt.float32
F32R = mybir.dt.float32r

B, T, D, H, Dh = 4, 2048, 1024, 16, 64
FC = 512          # features per core (8 heads)
NG = 2            # head groups per core (4 heads each)
FG = FC // NG     # 256 features per group
NTC = T // 512    # 4 t-chunks
NTT = T // 128    # 16 t-tiles
ND = D // 128     # 8 d-chunks


class _TC(tile.TileContext):
    """TileContext whose tail Drain carries at most one sem wait.

    The walrus build in this container rejects a Drain with >1 sync waits
    (setupSyncWait: "Too many sync wait commands"), so spread the waits over
    a chain of Drain instructions instead.
    """

    def _drain_and_barrier(self, tick_clock, wait_clock):
        drain_inst = self.nc.sync.drain()
        wait_clock.add_sem_waits(
            drain_inst.ins, ScopedClock({None: tick_clock.global_clock})
        )
        si = drain_inst.ins.sync_info
        if si is not None and len(si.on_wait) > 1:
            waits = list(si.on_wait)
            drain_inst.ins.sync_info = bass_rust.SyncInfo(
                on_wait=waits[:1], on_update=list(si.on_update)
            )
            for w in waits[1:]:
                d2 = self.nc.sync.drain()
                d2.ins.sync_info = bass_rust.SyncInfo(on_wait=[w], on_update=[])
        self.nc.all_engine_barrier()
        popped = self.nc._tile_sem_poison_stack.pop()
        assert popped is self._sem_poison
        self.nc.clear_and_free_semaphores(list(self.sems.allocated().values()))
        self.nc.all_engine_barrier()


def _r(ap):
    return ap.bitcast(F32R)


def _split_waits(nc, max_waits=1):
    """Hoist extra sem waits onto same-engine NoOps.

    The walrus build here allows only one sync wait on several instruction
    structs (Drain, the fp32/fp32r matmul LW struct). Engine queues are
    in-order, so moving waits to a preceding NoOp on the same engine is
    semantics-preserving.
    """
    n = 0
    for fn in nc.m.functions:
        for bb in fn.blocks:
            out = []
            for inst in bb.instructions:
                si = inst.sync_info
                if si is not None and len(si.on_wait) > max_waits:
                    waits = list(si.on_wait)
                    extra, keep = waits[:-max_waits], waits[-max_waits:]
                    for i, w in enumerate(extra):
                        nop = mybir.InstNoOp(
                            name=f"{inst.name}_ws{i}", engine=inst.engine
                        )
                        nop.sync_info = bass_rust.SyncInfo(on_wait=[w], on_update=[])
                        out.append(nop)
                        n += 1
                    inst.sync_info = bass_rust.SyncInfo(
                        on_wait=keep, on_update=list(si.on_update)
                    )
                out.append(inst)
            bb.instructions = out
    return n


def _build_program():
    from contextlib import ExitStack

    nc = bass.Bass()

    x = nc.dram_tensor("x", [T, D], F32, kind="ExternalInput")
    wq = nc.dram_tensor("wq", [D, FC], F32R, kind="ExternalInput")
    wk = nc.dram_tensor("wk", [D, FC], F32R, kind="ExternalInput")
    wv = nc.dram_tensor("wv", [D, FC], F32R, kind="ExternalInput")
    wo = nc.dram_tensor("wo", [FC, D], F32R, kind="ExternalInput")
    cos2 = nc.dram_tensor("cos2", [128, T], F32, kind="ExternalInput")
    sin2 = nc.dram_tensor("sin2", [128, T], F32, kind="ExternalInput")
    ident = nc.dram_tensor("ident", [128, 128], F32, kind="ExternalInput")
    mk0 = nc.dram_tensor("mk0", [128, 1024], F32, kind="ExternalInput")
    mk256 = nc.dram_tensor("mk256", [128, 1024], F32, kind="ExternalInput")
    ones8 = nc.dram_tensor("ones8", [128, 8], F32R, kind="ExternalInput")
    ones64 = nc.dram_tensor("ones64", [1, 64], F32R, kind="ExternalInput")
    out = nc.dram_tensor("out", [T, D], F32, kind="ExternalOutput")

    with _TC(nc) as tc, ExitStack() as ctx:
        consts = ctx.enter_context(tc.tile_pool(name="consts", bufs=1))
        psum = ctx.enter_context(tc.tile_pool(name="psum", bufs=4, space="PSUM"))
        dram = ctx.enter_context(tc.tile_pool(name="dram", bufs=4, space="DRAM"))
        persist = ctx.enter_context(tc.tile_pool(name="persist", bufs=1))

        ident_t = consts.tile([128, 128], F32)
        nc.sync.dma_start(ident_t[:], ident[:])
        ones64_t = consts.tile([1, 64], F32R)
        nc.sync.dma_start(ones64_t[:], ones64[:])
        mk0_t = consts.tile([128, 1024], F32)
        nc.sync.dma_start(mk0_t[:], mk0[:])
        mk256_t = consts.tile([128, 1024], F32)
        nc.sync.dma_start(mk256_t[:], mk256[:])
        cos_t = consts.tile([128, T], F32)
        nc.sync.dma_start(cos_t[:], cos2[:])
        sin_t = consts.tile([128, T], F32)
        nc.sync.dma_start(sin_t[:], sin2[:])
        # rotate_half sign fold: rows 0:32 / 64:96 get -sin
        nc.scalar.mul(sin_t[0:32, :], sin_t[0:32, :], -1.0)
        nc.scalar.mul(sin_t[64:96, :], sin_t[64:96, :], -1.0)

        # xT spill tiles (written during group 0, re-read by group 1)
        xt_dram = [dram.tile([D, 512], F32R, tag=f"xt{i}", name=f"xt_dram{i}") for i in range(NTC)]

        # attention outputs, persistent across both groups: 4 f-chunks [128, T]
        ao = [persist.tile([128, T], F32R, tag=f"ao{i}", name=f"ao{i}") for i in range(4)]
        # v (natural layout) with a ones column per head: 8 heads x 65 cols
        vt = [persist.tile([128, 8 * 65], F32R, tag=f"vt{i}", name=f"vt{i}") for i in range(NTT)]
        for i in range(NTT):
            nc.sync.dma_start(vt[i][:, 64::65], ones8[:])

        for g in range(NG):
            # per-group persistent activations (slots reused across groups)
            qt = [persist.tile([128, T], F32R, tag=f"qt{i}", name=f"qt{i}g{g}") for i in range(2)]
            kt = [persist.tile([128, T], F32R, tag=f"kt{i}", name=f"kt{i}g{g}") for i in range(2)]

            with ExitStack() as gctx:
                # ---- Phase B(g): projections + RoPE
                wp = gctx.enter_context(tc.tile_pool(name=f"w{g}", bufs=1))
                xtc = gctx.enter_context(tc.tile_pool(name=f"xtc{g}", bufs=2))
                rop = gctx.enter_context(tc.tile_pool(name=f"rope{g}", bufs=3))
                if g == 0:
                    xld = gctx.enter_context(tc.tile_pool(name="xload", bufs=4))

                gsl = slice(g * FG, (g + 1) * FG)
                wq_t = wp.tile([128, ND * FG], F32R, tag="wq")
                nc.sync.dma_start(
                    wq_t[:].rearrange("p (d f) -> p d f", d=ND),
                    wq[:, gsl].rearrange("(d p) f -> p d f", p=128),
                )
                wk_t = wp.tile([128, ND * FG], F32R, tag="wk")
                nc.sync.dma_start(
                    wk_t[:].rearrange("p (d f) -> p d f", d=ND),
                    wk[:, gsl].rearrange("(d p) f -> p d f", p=128),
                )
                wv_t = wp.tile([128, ND * FG], F32R, tag="wv")
                nc.sync.dma_start(
                    wv_t[:].rearrange("p (d f) -> p d f", d=ND),
                    wv[:, gsl].rearrange("(d p) f -> p d f", p=128),
                )

                for tcc in range(NTC):
                    xc = xtc.tile([128, ND * 512], F32R, tag="xc")
                    if g == 0:
                        # transpose x[tc] on the fly: load 4 t-tiles, PE-transpose
                        # each 128x128 block into xc, and spill xc for group 1
                        xsb = []
                        for q in range(4):
                            t0 = (tcc * 4 + q) * 128
                            xt_ = xld.tile([128, D], F32, tag="xl")
                            nc.sync.dma_start(xt_[:], x[t0 : t0 + 128, :])
                            xsb.append(xt_)
                        for dpair in range(ND // 2):
                            tp = psum.tile([128, 1024], F32, tag="pp")
                            for half in range(2):
                                d = dpair * 2 + half
                                for q in range(4):
                                    nc.tensor.transpose(
                                        tp[:, half * 512 + q * 128 : half * 512 + (q + 1) * 128],
                                        xsb[q][:, d * 128 : (d + 1) * 128],
                                        ident_t[:],
                                    )
                            for half in range(2):
                                d = dpair * 2 + half
                                nc.vector.tensor_copy(
                                    xc[:, d * 512 : (d + 1) * 512],
                                    tp[:, half * 512 : (half + 1) * 512],
                                )
                                nc.sync.dma_start(
                                    xt_dram[tcc][d * 128 : (d + 1) * 128, :],
                                    xc[:, d * 512 : (d + 1) * 512],
                                )
                    else:
                        nc.sync.dma_start(
                            xc[:].rearrange("p (d t) -> p d t", d=ND),
                            xt_dram[tcc][:].rearrange("(d p) t -> p d t", p=128),
                        )
                    tsl = slice(tcc * 512, (tcc + 1) * 512)
                    # q/k projections (transposed outputs) + RoPE
                    for dst, w_t in ((qt, wq_t), (kt, wk_t)):
                        ps = psum.tile([128, 1024], F32, tag="pp")
                        for fp in range(2):
                            for d in range(ND):
                                nc.tensor.matmul(
                                    ps[:, fp * 512 : fp * 512 + 512],
                                    w_t[:, d * FG + fp * 128 : d * FG + (fp + 1) * 128],
                                    xc[:, d * 512 : (d + 1) * 512],
                                    start=(d == 0),
                                    stop=(d == ND - 1),
                                )
                        for fp in range(2):
                            psl = ps[:, fp * 512 : fp * 512 + 512]
                            raw = rop.tile([128, 512], F32, tag="raw")
                            nc.scalar.copy(raw[:], psl)
                            rot = rop.tile([128, 512], F32, tag="rot")
                            for hb in range(2):
                                o = hb * 64
                                nc.sync.dma_start(rot[o : o + 32, :], raw[o + 32 : o + 64, :])
                                nc.sync.dma_start(rot[o + 32 : o + 64, :], raw[o : o + 32, :])
                            dtile = dst[fp]
                            nc.vector.tensor_mul(dtile[:, tsl], psl, cos_t[:, tsl])
                            nc.vector.tensor_mul(rot[:], rot[:], sin_t[:, tsl])
                            nc.vector.tensor_add(dtile[:, tsl], dtile[:, tsl], rot[:])
                    # v projection (natural layout, per-head 65-col strided store)
                    pv = psum.tile([128, 1024], F32, tag="pp")
                    for tb in range(4):
                        for d in range(ND):
                            nc.tensor.matmul(
                                pv[:, tb * 256 : tb * 256 + 256],
                                xc[:, d * 512 + tb * 128 : d * 512 + (tb + 1) * 128],
                                wv_t[:, d * FG : (d + 1) * FG],
                                start=(d == 0),
                                stop=(d == ND - 1),
                            )
                    for tb in range(4):
                        i = tcc * 4 + tb
                        vdst = vt[i][:, :].rearrange("p (h c) -> p h c", c=65)[
                            :, 4 * g : 4 * g + 4, 0:64
                        ]
                        vsrc = pv[:, tb * 256 : (tb + 1) * 256].rearrange(
                            "p (h c) -> p h c", c=64
                        )
                        nc.vector.tensor_copy(vdst, vsrc)

            # ---- Phase C(g): attention for the 4 heads of this group
            with ExitStack() as cctx:
                ep = cctx.enter_context(tc.tile_pool(name=f"exp{g}", bufs=3))
                rp = cctx.enter_context(tc.tile_pool(name=f"rcp{g}", bufs=3))
                for hl in range(4):
                    fp = hl // 2
                    o = (hl % 2) * 64
                    for tcc in range(NTC):
                        tsl = slice(tcc * 512, (tcc + 1) * 512)
                        av_ps = psum.tile([128, 1024], F32, tag="pp")
                        av = av_ps[0:65, 0:512]
                        ngrp = 2 * tcc + 2
                        for g2 in range(ngrp):
                            sc = psum.tile([128, 1024], F32, tag="pp")
                            for half in range(2):
                                si = 2 * g2 + half
                                nc.tensor.matmul(
                                    sc[:, half * 512 : half * 512 + 512],
                                    kt[fp][o : o + 64, si * 128 : (si + 1) * 128],
                                    qt[fp][o : o + 64, tsl],
                                    start=True,
                                    stop=True,
                                )
                            ex = ep.tile([128, 1024], F32R, tag="ex")
                            nc.scalar.activation(
                                ex[:], sc[:], mybir.ActivationFunctionType.Exp, scale=0.125
                            )
                            if g2 >= 2 * tcc:
                                # diagonal group: zero the s > t region post-exp
                                mt = mk0_t if g2 == 2 * tcc else mk256_t
                                nc.vector.tensor_mul(ex[:], ex[:], mt[:])
                            for half in range(2):
                                si = 2 * g2 + half
                                nc.tensor.matmul(
                                    av,
                                    vt[si][:, (4 * g + hl) * 65 : (4 * g + hl) * 65 + 65],
                                    ex[:, half * 512 : half * 512 + 512],
                                    start=(g2 == 0 and half == 0),
                                    stop=(g2 == ngrp - 1 and half == 1),
                                )
                        av_sb = rp.tile([65, 512], F32, tag="avs")
                        nc.vector.tensor_copy(av_sb[:], av_ps[0:65, 0:512])
                        rcp = rp.tile([1, 512], F32R, tag="rc")
                        with nc.allow_low_precision(reason="f32r recip for PE bcast"):
                            nc.vector.reciprocal(rcp[:], av_sb[64:65, :])
                        pb = psum.tile([128, 1024], F32, tag="pp", name="pb")
                        nc.tensor.matmul(
                            pb[0:64, 0:512], ones64_t[:], rcp[:], start=True, stop=True
                        )
                        rb = rp.tile([64, 512], F32, tag="rb")
                        nc.scalar.copy(rb[:], pb[0:64, 0:512])
                        nc.vector.tensor_mul(
                            ao[2 * g + fp][o : o + 64, tsl],
                            av_sb[0:64, :],
                            rb[:],
                        )

        # ---- Phase D: output projection (row-sharded Wo partial)
        with tc.tile_pool(name="wo", bufs=1) as wop, tc.tile_pool(
            name="oev", bufs=3
        ) as oev:
            wo_t = wop.tile([128, 4 * D], F32R, tag="wo")
            nc.sync.dma_start(
                wo_t[:].rearrange("p (c o) -> p c o", c=4),
                wo[:].rearrange("(c p) o -> p c o", p=128),
            )
            for i in range(NTT):
                po = psum.tile([128, 1024], F32, tag="pp")
                for n in range(2):
                    for c in range(4):
                        nc.tensor.matmul(
                            po[:, n * 512 : n * 512 + 512],
                            ao[c][:, i * 128 : (i + 1) * 128],
                            wo_t[:, c * D + n * 512 : c * D + n * 512 + 512],
                            start=(c == 0),
                            stop=(c == 3),
                        )
                oe = oev.tile([128, 1024], F32, tag="oe")
                nc.vector.tensor_copy(oe[:], po[:])
                nc.sync.dma_start(out[i * 128 : (i + 1) * 128, :], oe[:])

    _split_waits(nc)
    return nc


_NC_CACHE = None


def _get_nc():
    global _NC_CACHE
    if _NC_CACHE is None:
        _NC_CACHE = _build_program()
    return _NC_CACHE


def _consts():
    j = np.arange(1024)
    p = np.arange(128)
    s_rel = p[:, None] + 128 * (j[None, :] // 512)  # s offset within group
    t_rel = j[None, :] % 512
    return {
        "ident": np.eye(128, dtype=np.float32),
        "mk0": (s_rel <= t_rel).astype(np.float32),
        "mk256": (s_rel + 256 <= t_rel).astype(np.float32),
        "ones8": np.ones((128, 8), dtype=np.float32),
        "ones64": np.ones((1, 64), dtype=np.float32),
    }


def kernel(x, cos, sin, Wq, Wk, Wv, Wo):
    from concourse.bass_utils import run_bass_kernel_spmd

    x = np.asarray(x, dtype=np.float32)
    cos = np.asarray(cos, dtype=np.float32)
    sin = np.asarray(sin, dtype=np.float32)
    Wq = np.asarray(Wq, dtype=np.float32)
    Wk = np.asarray(Wk, dtype=np.float32)
    Wv = np.asarray(Wv, dtype=np.float32)
    Wo = np.asarray(Wo, dtype=np.float32)

    cos2 = np.ascontiguousarray(np.tile(cos.T, (2, 1)))  # [128, T]
    sin2 = np.ascontiguousarray(np.tile(sin.T, (2, 1)))
    consts = _consts()

    in_maps = []
    for c in range(8):
        b, hh = c // 2, c % 2
        sl = slice(hh * FC, (hh + 1) * FC)
        in_maps.append(
            {
                "x": np.ascontiguousarray(x[b]),
                "wq": np.ascontiguousarray(Wq[:, sl]),
                "wk": np.ascontiguousarray(Wk[:, sl]),
                "wv": np.ascontiguousarray(Wv[:, sl]),
                "wo": np.ascontiguousarray(Wo[sl, :]),
                "cos2": cos2,
                "sin2": sin2,
                **consts,
            }
        )

    nc = _get_nc()
    res = run_bass_kernel_spmd(nc, in_maps, core_ids=list(range(8)))
    outs = [res.results[c]["out"] for c in range(8)]
    full = np.stack([outs[2 * b] + outs[2 * b + 1] for b in range(B)])
    return full.astype(np.float32)
